# revision 24
# baseline (speedup 1.0000x reference)
"""Trainium2 Bass kernel for nn_BiasedConLoss (supervised-contrastive biased loss).

Math (validated against the jax reference to ~2e-6 rel):
  X = concat(features, features_cr)            [M=8192, D=256], rows L2-normalized
  A = X @ X.T  (raw dots), sims = A / T
  row max = diag(A)/T (diag==1 dominates off-diag cos sims)
  The only O(M^2) quantity needed is Q_i = sum_j exp((A_ij - 1)/T).
  Everything else (pos_dot via matvec, partner dots, diag, P/U) is O(M*D)
  and computed on host in float64.

Device (8 NeuronCores, SPMD):
  Each core owns 1024 rows of A. It gets xin [256, 9216] =
  per k-half (128 d's): [ XT_k cols 0:512 | XT_k cols g0:g0+1024 (own rows) |
                          XT_k cols 512:8192 ]
  The "bigtile" (first 1536 cols) holds both the first rhs col-tile and all
  lhsT columns, so the first matmul of each k-group waits on ONE DMA
  semaphore (walrus allows only one sync-wait per instruction here).
  Per row tile r (128 rows) and col chunk c (2048 cols): 8 f32r matmuls into
  a [128,2048] PSUM tile (2 k-chunks x 4 col tiles), then one ScalarE
  activation Exp(in*1/T - 1/T) in-place with accum_out giving the row-sums.
  Output stats [128, 32]: partial row sums per (chunk, row tile).
"""
import numpy as np

import concourse.bass as bass
import concourse.tile as tile
from concourse import mybir
from concourse.bass_utils import run_bass_kernel_spmd
from concourse.vector_clock import ScopedClock, VectorClock

F32 = mybir.dt.float32
F16 = mybir.dt.float16

T = 0.07
N = 4096
D = 256
M = 2 * N           # 8192
NCORES = 8
ROWS_PER_CORE = M // NCORES          # 1024
NR = ROWS_PER_CORE // 128            # 8 row tiles per core
NJ = M // 512                        # 16 col tiles of 512
NT = 4                               # col tiles per chunk
NCHUNK = NJ // NT                    # 4 chunks of 2048 cols
XIN_COLS = 512 + ROWS_PER_CORE + (M - 512)   # 9216


_SELF_SEM_PREFIX = {
    mybir.EngineType.PE: "PE_",
    mybir.EngineType.Activation: "Activation_",
    mybir.EngineType.DVE: "DVE_",
}


class _SplitDrainTileContext(tile.TileContext):
    """Two walrus-compat adjustments for this toolchain (which allows only
    ONE sync-wait per instruction):

    1. Strip same-engine semaphore self-waits from PE/ACT/DVE instructions.
       These engines execute and complete their instruction streams strictly
       in order (PE matmuls are pc-monotone; ACT/DVE are strict-FIFO queues),
       so a wait on the engine's own completion semaphore is redundant with
       program order. Tile emits them conservatively for WAW/WAR hazards
       across PSUM-slot reuse.
    2. Split the kernel-tail drain's sem waits across many Drain
       instructions."""

    def _lower_ordered_insts(self, postordered_blocks):
        for insts in postordered_blocks.values():
            for inst in insts:
                si = getattr(inst, "sync_info", None)
                if si is None or not si.on_wait:
                    continue
                prefix = _SELF_SEM_PREFIX.get(inst.engine)
                kept = si.on_wait
                if prefix is not None:
                    kept = [
                        w for w in kept
                        if not (w.ant_name or "").startswith(prefix)
                    ]
                if (
                    inst.engine == mybir.EngineType.Pool
                    and type(inst).__name__ == "InstDMACopy"
                ):
                    # The only Pool DMA here is the stats store, whose sole
                    # data dep is the DVE-written stats tile; DMASW waits on
                    # it are same-queue FIFO ordering (redundant in-order).
                    kept = [
                        w for w in kept
                        if not (w.ant_name or "").startswith("DMASW")
                    ]
                if len(kept) != len(si.on_wait):
                    si.on_wait = kept
        return super()._lower_ordered_insts(postordered_blocks)

    def _drain_and_barrier(self, tick_clock, wait_clock):
        full = tick_clock.global_clock
        n = len(full)
        procs = [p for p in range(n) if full[p] > 0]
        for p in procs:
            vec = [full[q] if q == p else 0 for q in range(n)]
            d = self.nc.sync.drain()
            wait_clock.add_sem_waits(d.ins, ScopedClock({None: VectorClock(vec)}))
        if not procs:
            d = self.nc.sync.drain()
            wait_clock.add_sem_waits(
                d.ins, ScopedClock({None: tick_clock.global_clock})
            )
        self.nc.all_engine_barrier()
        assert self.sems is not None
        popped = self.nc._tile_sem_poison_stack.pop()
        assert popped is self._sem_poison
        self.nc.clear_and_free_semaphores(list(self.sems.allocated().values()))
        self.nc.all_engine_barrier()


def _build():
    nc = bass.Bass("TRN2", target_bir_lowering=False, debug=False,
                   num_swdge_queues=1)
    xin = nc.dram_tensor("xin", [2 * 128, XIN_COLS], F16, kind="ExternalInput").ap()
    stats = nc.dram_tensor(
        "stats", [128, NCHUNK * NR + 1], F32, kind="ExternalOutput"
    ).ap()

    bias_t = nc.alloc_sbuf_tensor("bias_const", [128, 1], F32)
    warm_t = nc.alloc_sbuf_tensor("warm_zeros", [128, 512], F16)

    with _SplitDrainTileContext(nc) as tc:
        ones = nc.const_aps.tensor(1.0, (128, 1), mybir.dt.float32)
        nc.scalar.mul(bias_t.ap(), ones, -1.0 / T)
        nc.vector.memset(warm_t.ap(), 0.0)
        with tc.tile_pool(name="big", bufs=2) as big_pool, \
             tc.tile_pool(name="rhs", bufs=2) as rhs_pool, \
             tc.tile_pool(name="acc", bufs=NCHUNK * NR + 1) as acc_pool, \
             tc.tile_pool(name="stat", bufs=1) as stat_pool, \
             tc.tile_pool(name="ps", bufs=2, space="PSUM") as ps_pool:

            # bigA per k-half: [rhs j0 (512) | lhsT r0 (128)]; bigB: lhsT r1..r7.
            # Packing lhsT with rhs j0 keeps the first matmul of each k-group
            # down to ONE DMA-sem wait; the split keeps the gating transfer
            # small so PE starts early.
            bigA, bigB = [], []
            rhs = {}

            # DMA dispatch costs ~0.6us per dma_start, and each engine's
            # dynamic HWDGE ring runs its transfers sequentially. Inputs ride
            # in 12 chunk-aligned transfers split across TWO rings (sync +
            # vector engines), with chunk 0's operands first on the sync ring
            # so compute starts ~15us in while the rest streams.
            R = {}          # (c, k) -> tile holding that chunk's rhs cols
            L0 = 512 + ROWS_PER_CORE

            def dma_R(c, k, eng, tag=None, s0=None, w=None):
                w = (4 * 512 if w is None else w)
                s0 = (L0 + (4 * c - 1) * 512 if s0 is None else s0)
                bt = rhs_pool.tile([128, w], F16, tag=tag or f"R{c}")
                eng.dma_start(
                    out=bt[:], in_=xin[128 * k:128 * (k + 1), s0:s0 + w]
                )
                R[(c, k)] = bt

            rings = [nc.sync, nc.scalar]
            # chunk 0's j1..j3 split so the very first exp (j0,j1 halves of
            # row tile 0) is gated by only ~0.57MB of DMA.
            R0a, R0b = {}, {}
            for k in range(2):
                bt = big_pool.tile([128, 640], F16, tag="bigA")
                rings[k].dma_start(out=bt[:], in_=xin[128 * k:128 * (k + 1), 0:640])
                bigA.append(bt)
            for k in range(2):
                bt = rhs_pool.tile([128, 512], F16, tag="R0a")
                rings[k].dma_start(
                    out=bt[:], in_=xin[128 * k:128 * (k + 1), L0:L0 + 512]
                )
                R0a[k] = bt
            for k in range(2):
                bt = big_pool.tile([128, ROWS_PER_CORE - 128], F16, tag="bigB")
                rings[k].dma_start(
                    out=bt[:],
                    in_=xin[128 * k:128 * (k + 1), 640:512 + ROWS_PER_CORE],
                )
                bigB.append(bt)
            for k in range(2):
                bt = rhs_pool.tile([128, 1024], F16, tag="R0b")
                rings[k].dma_start(
                    out=bt[:], in_=xin[128 * k:128 * (k + 1), L0 + 512:L0 + 1536]
                )
                R0b[k] = bt
            dma_R(1, 0, nc.sync)
            dma_R(1, 1, nc.sync)
            dma_R(2, 0, nc.sync)
            dma_R(2, 1, nc.sync)
            dma_R(3, 0, nc.sync)
            dma_R(3, 1, nc.sync)

            def rhs_ap(k, j):
                if j == 0:
                    return bigA[k][:, 0:512]
                if j == 1:
                    return R0a[k][:]
                if j in (2, 3):
                    return R0b[k][:, 512 * (j - 2):512 * (j - 1)]
                c = j // NT
                return R[(c, k)][:, 512 * (j - 4 * c):512 * (j - 4 * c + 1)]

            def lhsT_ap(k, r):
                if r == 0:
                    return bigA[k][:, 512:640]
                return bigB[k][:, 128 * (r - 1):128 * r]

            stat_sb = stat_pool.tile([128, NCHUNK * NR + 1], F32)

            # PE warm-up: ~3.5us of zero matmuls on a preamble-initialized
            # const tile. No input deps -> starts immediately, releases the
            # HAM clock throttle before the first real matmul arrives.
            warm_ap = warm_t.ap()
            ps_warm = ps_pool.tile([128, 512 * NT], F32, tag="ps")
            for _ in range(4):
                nc.tensor.matmul(
                    ps_warm[0:1, 0:512],
                    lhsT=warm_ap[:, 0:1],
                    rhs=warm_ap[:],
                    start=True, stop=True,
                    skip_group_check=True,
                )

            def do_act(ps_ap, col):
                acc = acc_pool.tile([128, 1], F32)
                nc.scalar.activation(
                    out=ps_ap, in_=ps_ap,
                    func=mybir.ActivationFunctionType.Exp,
                    bias=bias_t.ap(), scale=1.0 / T,
                    accum_out=acc[:],
                )
                nc.vector.tensor_copy(stat_sb[:, col:col + 1], acc[:])

            # (c0, r0) fast start: two half-width exps so the first one only
            # needs j0/j1 of both k-halves (~0.57MB of DMA).
            ps0 = ps_pool.tile([128, 512 * NT], F32, tag="ps")
            for half in range(2):
                for k in range(2):
                    for t in (2 * half, 2 * half + 1):
                        nc.tensor.matmul(
                            ps0[:, 512 * t:512 * (t + 1)],
                            lhsT=lhsT_ap(k, 0),
                            rhs=rhs_ap(k, t),
                            start=(k == 0), stop=(k == 1),
                        )
                do_act(ps0[:, 1024 * half:1024 * (half + 1)],
                       NCHUNK * NR if half == 0 else 0)

            # Column-chunk-outer: chunk c only needs its 8 input tiles, so
            # chunk 0's compute (~18us) hides the remaining input DMA.
            for c in range(NCHUNK):
                for r in range(NR):
                    if c == 0 and r == 0:
                        continue
                    ps = ps_pool.tile([128, 512 * NT], F32)
                    # WAR-absorber: the first MM of a new chunk would need
                    # BOTH the PSUM-reuse WAR sem and a fresh DMA sem ->
                    # 2 waits (walrus allows 1). Absorb the WAR wait with one
                    # matmul on already-observed tiles; the real k0 group
                    # overwrites it (start=True).
                    if c > 0 and r == 0:
                        nc.tensor.matmul(
                            ps[:, 0:512],
                            lhsT=lhsT_ap(0, 0),
                            rhs=rhs_ap(0, 0),
                            start=True, stop=True,
                            skip_group_check=True,
                        )
                    for k in range(2):
                        lhsT = lhsT_ap(k, r)
                        for t in range(NT):
                            j = NT * c + t
                            nc.tensor.matmul(
                                ps[:, 512 * t:512 * (t + 1)],
                                lhsT=lhsT,
                                rhs=rhs_ap(k, j),
                                start=(k == 0), stop=(k == 1),
                                skip_group_check=(c > 0 and r == 0),
                            )
                    do_act(ps[:], c * NR + r)
            # SWDGE: keeps the output DMA off the busy HWDGE queues so it
            # carries only the DVE wait (1-wait limit).
            nc.gpsimd.dma_start(out=stats[:], in_=stat_sb[:])
    return nc


_NC_CACHE = None


def _get_nc():
    global _NC_CACHE
    if _NC_CACHE is None:
        _NC_CACHE = _build()
    return _NC_CACHE


def kernel(labels, all_features, all_features_cr, _trace=False):
    labels = np.asarray(labels)
    f = np.asarray(all_features, dtype=np.float32)
    f_cr = np.asarray(all_features_cr, dtype=np.float32)

    # fp16 GEMM on device: products of fp16 values are exact in the fp32
    # PSUM accumulator, so host float64 math over the SAME fp16-rounded
    # values matches the device GEMM to fp32-accumulation noise.
    X16 = np.concatenate([f, f_cr], axis=0).astype(np.float16)   # [M, D]
    X32 = X16.astype(np.float32)
    XT = np.ascontiguousarray(X16.T)                       # [D, M] fp16

    in_maps = []
    for c in range(NCORES):
        g0 = c * ROWS_PER_CORE
        xin = np.empty((2 * 128, XIN_COLS), dtype=np.float16)
        for k in range(2):
            rows = slice(128 * k, 128 * (k + 1))
            xin[rows, 0:512] = XT[rows, 0:512]
            xin[rows, 512:512 + ROWS_PER_CORE] = XT[rows, g0:g0 + ROWS_PER_CORE]
            xin[rows, 512 + ROWS_PER_CORE:] = XT[rows, 512:M]
        in_maps.append({"xin": xin})

    nc = _get_nc()
    res = run_bass_kernel_spmd(
        nc, in_maps, core_ids=list(range(NCORES)), trace=_trace
    )
    kernel.last_exec_time_ns = res.exec_time_ns
    kernel.last_trace = res.instructions_and_trace

    # stats[p, c*NR + r] = sum_{j in chunk c} exp((A[g0+128r+p, j] - 1)/T)
    Q = np.empty(M, dtype=np.float64)
    for core in range(NCORES):
        st = res.results[core]["stats"].astype(np.float64)   # [128, 33]
        per_row = st[:, :NCHUNK * NR].reshape(128, NCHUNK, NR).sum(axis=1)
        per_row[:, 0] += st[:, NCHUNK * NR]   # (c0,r0) first-half partial
        for r in range(NR):
            i0 = core * ROWS_PER_CORE + r * 128
            Q[i0:i0 + 128] = per_row[:, r]

    # ---- host epilogue (float64, O(M*D)) ----
    X = X32.astype(np.float64)
    lab = np.asarray(labels)
    all_labels = np.concatenate([lab, lab]).astype(np.float64)
    pos_f = (all_labels == 1).astype(np.float64)
    neg_f = 1.0 - pos_f
    P = pos_f.sum()
    U = neg_f.sum()

    d = np.sum(X * X, axis=1)                 # diag of A
    row_sum = Q * np.exp((1.0 - d) / T)       # = 1 + sum_{j!=i} exp((A_ij-d_i)/T)
    row_logsum = np.log(row_sum)

    w_pos = pos_f @ X
    pos_dot_raw = X @ w_pos
    spos = (pos_dot_raw - P * d) / T
    sup_row = spos - M * row_logsum
    loss_sup = np.sum(pos_f * (-sup_row / P)) / P

    partner = np.sum(X * np.roll(X, -N, axis=0), axis=1)
    unsup_row = (partner - d) / T - M * row_logsum
    loss_unsup = np.sum(neg_f * (-unsup_row / U)) / U

    return (np.float32(loss_sup), np.float32(loss_unsup))
